# revision 34
# baseline (speedup 1.0000x reference)
"""Trainium2 Bass kernel: causal multi-head self-attention block (B=8, T=1024, E=768, H=12).

Sharding: data-parallel over batch — one batch element per NeuronCore, 8 cores,
no collectives. Each core computes the full attention block for its batch row.

Self-contained: hardcodes all shapes; only imports concourse (installed system-wide).
"""

import numpy as np

B, T, E, H, Dh = 8, 1024, 768, 12, 64
F3 = 3 * E            # 2304
KC = E // 128         # 6 e-chunks
MT = T // 128         # 8 t-tiles
NPAIR = H // 2        # 6 head pairs
SCALE = 1.0 / float(np.sqrt(Dh))

_NC_CACHE = None


def build_nc():
    import concourse.mybir as mybir
    from concourse import bacc
    from concourse.tile import TileContext
    from concourse.masks import make_identity

    bf = mybir.dt.bfloat16
    f32 = mybir.dt.float32
    COPY = mybir.ActivationFunctionType.Copy
    EXP = mybir.ActivationFunctionType.Exp
    LN = mybir.ActivationFunctionType.Ln
    ACT_SET_LN_EXP = 6  # natural_log_exp_and_others: holds both Ln and Exp

    nc = bacc.Bacc("TRN2", target_bir_lowering=False, debug=False, num_devices=B, name="attn_dp")

    X_ext = nc.declare_dram_parameter("X", [T, E], f32, isOutput=False)
    W1_ext = nc.declare_dram_parameter("W1", [E, F3], f32, isOutput=False)
    b1_ext = nc.declare_dram_parameter("b1", [F3], f32, isOutput=False)
    W2_ext = nc.declare_dram_parameter("W2", [E, E], f32, isOutput=False)
    b2_ext = nc.declare_dram_parameter("b2", [E], f32, isOutput=False)
    out_ext = nc.declare_dram_parameter("out", [T, E], f32, isOutput=True)

    with TileContext(nc) as tc:
        with (
            tc.tile_pool(name="persist", bufs=1) as persist,
            tc.tile_pool(name="stage", bufs=2) as stage,
            tc.tile_pool(name="ptpool", bufs=10) as ptpool,
            tc.tile_pool(name="recpool", bufs=3) as recpool,
            tc.tile_pool(name="opool", bufs=2) as opool,
            tc.tile_pool(name="ps_score", bufs=1, space="PSUM") as ps_score,
            tc.tile_pool(name="ps_z", bufs=2, space="PSUM") as ps_z,
            tc.tile_pool(name="ps_acc", bufs=2, space="PSUM") as ps_acc,
        ):
            # One activation-table load for the whole kernel (covers Exp + Ln + Copy);
            # bacc's insert_act_table_loads sees coverage and adds none.
            nc.scalar.add_instruction(mybir.InstLoadActFuncSet(
                name=nc.get_next_instruction_name(), ins=[], outs=[],
                act_func_set_id=ACT_SET_LN_EXP))

            # ---- constants ----
            ident = persist.tile([128, 128], bf, tag="ident")
            make_identity(nc, ident[:])
            # multiplicative causal mask for the diagonal 128x128 block:
            # mask[k, q] = 1 where q >= k else 0
            diagmask = persist.tile([128, 128], bf, tag="diagmask")
            nc.gpsimd.memset(diagmask[:], 1.0)
            nc.gpsimd.affine_select(
                out=diagmask[:], in_=diagmask[:],
                compare_op=mybir.AluOpType.is_ge, fill=0.0, base=0,
                pattern=[[1, 128]], channel_multiplier=-1,
            )

            # per-partition bias for Q/K part of b1: b1qk[p, m] = b1[m*128 + p]
            b1qk = persist.tile([128, 12], f32, tag="b1qk")
            nc.sync.dma_start(
                out=b1qk[:], in_=b1_ext[0:1536].rearrange("(m p) -> p m", p=128)
            )
            # row biases, pre-broadcast across partitions (folded into DVE copies)
            b1v_f = stage.tile([1, E], f32, tag="rowstage")
            nc.sync.dma_start(out=b1v_f[:], in_=b1_ext[None, 1536:2304])
            b1vb = persist.tile([128, E], f32, tag="b1vb")
            nc.gpsimd.partition_broadcast(b1vb[:], b1v_f[:])
            b2_f = stage.tile([1, E], f32, tag="rowstage")
            nc.sync.dma_start(out=b2_f[:], in_=b2_ext[None, :])
            b2b = persist.tile([128, E], f32, tag="b2b")
            nc.gpsimd.partition_broadcast(b2b[:], b2_f[:])

            # ---- X: DMA f32, cast bf16 (DVE), then per-block DMA-transpose to XT[e, t] ----
            XT = persist.tile([128, KC, T], bf, tag="XT")

            def xload(mt):
                xst = stage.tile([128, E], f32, tag="xstage")
                nc.sync.dma_start(out=xst[:], in_=X_ext[mt * 128:(mt + 1) * 128, :])
                xbf = stage.tile([128, E], bf, tag="xbf")
                nc.scalar.activation(xbf[:], xst[:], COPY)
                for eg in range(KC // 2):  # pairs of e-chunks share one PSUM tile
                    pt_ps = ps_acc.tile([128, 256], bf, tag="acc")
                    nc.tensor.matmul(
                        pt_ps[:, 0:128], xbf[:, (2 * eg) * 128:(2 * eg + 1) * 128],
                        ident[:], is_transpose=True, start=True, stop=False,
                        skip_group_check=True)
                    nc.tensor.matmul(
                        pt_ps[:, 128:256], xbf[:, (2 * eg + 1) * 128:(2 * eg + 2) * 128],
                        ident[:], is_transpose=True, start=False, stop=True,
                        skip_group_check=True)
                    nc.scalar.activation(
                        XT[:, 2 * eg:2 * eg + 2, mt * 128:(mt + 1) * 128],
                        pt_ps[:].rearrange("p (a n) -> p a n", a=2),
                        COPY)

            for mt in range(MT):
                xload(mt)

            # ---- W1: DMA f32, cast to bf16 (spread across DVE/GPSIMD/ACT) ----
            W1bf = persist.tile([128, KC, F3], bf, tag="W1bf")
            for kc in range(KC):
                wst = stage.tile([128, F3], f32, tag="wstage")
                nc.sync.dma_start(out=wst[:], in_=W1_ext[kc * 128:(kc + 1) * 128, :])
                if kc % 2 == 0:
                    nc.vector.tensor_copy(W1bf[:, kc, :], wst[:])
                else:
                    nc.scalar.activation(W1bf[:, kc, :], wst[:], COPY)



            # QK[p, m, t]: m 0..5 = Q^T blocks (f rows m*128..), m 6..11 = K^T blocks
            QK = persist.tile([128, 12, T], bf, tag="QK")

            def qk_mtile(m):
                for nh in range(2):
                    ps = ps_acc.tile([128, 512], f32, tag="acc")
                    for kc in range(KC):
                        nc.tensor.matmul(
                            ps[:],
                            W1bf[:, kc, m * 128:(m + 1) * 128],
                            XT[:, kc, nh * 512:(nh + 1) * 512],
                            start=(kc == 0),
                            stop=(kc == KC - 1),
                        )
                    # bias (per-partition) + cast to bf16
                    nc.vector.tensor_scalar_add(
                        QK[:, m, nh * 512:(nh + 1) * 512], ps[:], b1qk[:, m:m + 1]
                    )

            qk_mtile(0)
            qk_mtile(6)

            # ---- V projection into V_aug[t-part, kt, h, 0:64] with ones col at 64 ----
            Vg = persist.tile([128, MT, H, Dh + 1], bf, tag="Vg")
            for mt in range(MT):
                nc.gpsimd.memset(Vg[:, mt, :, Dh:Dh + 1], 1.0)

            def vproj():
                for mt in range(MT):
                    for g, (n0, n) in enumerate(((0, 512), (512, 256))):
                        ps = ps_acc.tile([128, 512], f32, tag="acc")
                        for kc in range(KC):
                            nc.tensor.matmul(
                                ps[:, 0:n],
                                XT[:, kc, mt * 128:(mt + 1) * 128],
                                W1bf[:, kc, 1536 + n0:1536 + n0 + n],
                                start=(kc == 0),
                                stop=(kc == KC - 1),
                            )
                        ng = n // Dh
                        # bias add (pre-broadcast row) + cast to bf16, strided into V_aug
                        nc.vector.tensor_add(
                            Vg[:, mt, 8 * g:8 * g + ng, 0:Dh],
                            ps[:, 0:n].rearrange("p (h d) -> p h d", d=Dh),
                            b1vb[:, n0:n0 + n].rearrange("p (h d) -> p h d", d=Dh),
                        )

            # ---- attention + Z^T (divided) ----
            ZT = persist.tile([128, NPAIR, T], bf, tag="ZT")
            for hp in range(NPAIR):
                # scores + exp + mask for all kt; retain P^T tiles
                pts = []
                for kt in range(MT):
                    L = T - kt * 128
                    sc = ps_score.tile([128, 2, 1024], f32, tag="score")
                    for h01 in range(2):
                        base = h01 * 64
                        for c_off in range(0, L, 512):
                            n = min(512, L - c_off)
                            nc.tensor.matmul(
                                sc[:, h01, c_off:c_off + n],
                                QK[base:base + 64, 6 + hp, kt * 128:(kt + 1) * 128],
                                QK[base:base + 64, hp, kt * 128 + c_off:kt * 128 + c_off + n],
                                start=True,
                                stop=True,
                            )
                    ptile = ptpool.tile([128, 2, 1024], bf, tag="pt")
                    nc.scalar.activation(ptile[:, :, 0:L], sc[:, :, 0:L], EXP, scale=SCALE)
                    # causal mask on the diagonal 128x128 block (cols 0:128 of this
                    # tile): multiply by the precomputed 0/1 mask (DVE, bf16 2x)
                    for h01 in range(2):
                        nc.vector.tensor_mul(
                            ptile[:, h01, 0:128], ptile[:, h01, 0:128], diagmask[:]
                        )
                    pts.append(ptile)

                if hp == 0:
                    # V projection overlaps pair-0 exp on ACT
                    vproj()
                # emit next pair's QK projection here: overlaps with this pair's AV on PE
                if hp + 1 < NPAIR:
                    qk_mtile(hp + 1)
                    qk_mtile(6 + hp + 1)

                # attn @ V_aug, per q-chunk; denominator rides in row 64
                for c in range(2):
                    zs = []
                    for h01 in range(2):
                        z = ps_z.tile([128, 512], f32, tag="z")
                        zs.append(z)
                        h = 2 * hp + h01
                        kts = list(range(0, min(MT, 4 * (c + 1))))
                        for kt in kts:
                            zoff = max(kt * 128 - c * 512, 0)
                            n = 512 - zoff
                            poff = max(c * 512 - kt * 128, 0)
                            nc.tensor.matmul(
                                z[0:Dh + 1, zoff:zoff + n],
                                Vg[:, kt, h, 0:Dh + 1],
                                pts[kt][:, h01, poff:poff + n],
                                start=(kt == kts[0]),
                                stop=(kt == kts[-1]),
                            )
                    for h01 in range(2):
                        z = zs[h01]
                        # 1/den = exp(-ln(den)) on ScalarE; bf16 reciprocal row
                        lnden = recpool.tile([1, 512], f32, tag="lnden")
                        nc.scalar.activation(lnden[:], z[Dh:Dh + 1, :], LN)
                        rec = recpool.tile([1, 512], bf, tag="rec")
                        nc.scalar.activation(rec[:], lnden[:], EXP, scale=-1.0)
                        bc = recpool.tile([64, 512], bf, tag="bc")
                        nc.gpsimd.partition_broadcast(bc[:], rec[:])
                        nc.vector.tensor_tensor(
                            ZT[h01 * 64:(h01 + 1) * 64, hp, c * 512:(c + 1) * 512],
                            z[0:Dh, :],
                            bc[:],
                            op=mybir.AluOpType.mult,
                        )

            # ---- W2: loaded late (only needed for out-proj; overlaps attention) ----
            W2bf = persist.tile([128, KC, E], bf, tag="W2bf")
            for kc in range(KC):
                wst = stage.tile([128, F3], f32, tag="wstage")
                nc.sync.dma_start(out=wst[:, 0:E], in_=W2_ext[kc * 128:(kc + 1) * 128, :])
                if kc % 2 == 0:
                    nc.vector.tensor_copy(W2bf[:, kc, :], wst[:, 0:E])
                else:
                    nc.scalar.activation(W2bf[:, kc, :], wst[:, 0:E], COPY)

            # ---- output projection: Out[t, :] = Z^T.T @ W2 + b2 ----
            for mt in range(MT):
                osb = opool.tile([128, E], f32, tag="osb")
                for g, (n0, n) in enumerate(((0, 512), (512, 256))):
                    ps = ps_acc.tile([128, 512], f32, tag="acc")
                    for pc in range(KC):
                        nc.tensor.matmul(
                            ps[:, 0:n],
                            ZT[:, pc, mt * 128:(mt + 1) * 128],
                            W2bf[:, pc, n0:n0 + n],
                            start=(pc == 0),
                            stop=(pc == KC - 1),
                        )
                    # bias add (pre-broadcast row), f32
                    nc.vector.tensor_add(osb[:, n0:n0 + n], ps[:, 0:n], b2b[:, n0:n0 + n])
                nc.sync.dma_start(out=out_ext[mt * 128:(mt + 1) * 128, :], in_=osb[:])

    nc.compile()
    return nc


def _get_nc():
    global _NC_CACHE
    if _NC_CACHE is None:
        _NC_CACHE = build_nc()
    return _NC_CACHE


def _in_maps(X, W1, b1, W2, b2):
    X = np.ascontiguousarray(np.asarray(X, dtype=np.float32))
    W1 = np.ascontiguousarray(np.asarray(W1, dtype=np.float32))
    b1 = np.ascontiguousarray(np.asarray(b1, dtype=np.float32))
    W2 = np.ascontiguousarray(np.asarray(W2, dtype=np.float32))
    b2 = np.ascontiguousarray(np.asarray(b2, dtype=np.float32))
    assert X.shape == (B, T, E)
    return [
        {"X": X[i], "W1": W1, "b1": b1, "W2": W2, "b2": b2}
        for i in range(B)
    ]


def kernel(X, W1, b1, W2, b2):
    from concourse.bass_utils import run_bass_kernel_spmd

    nc = _get_nc()
    res = run_bass_kernel_spmd(nc, _in_maps(X, W1, b1, W2, b2), core_ids=list(range(B)))
    return np.stack([res.results[i]["out"] for i in range(B)]).astype(np.float32)


def kernel_traced(X, W1, b1, W2, b2, tmpdir=None):
    """Like kernel() but with neuron-profile tracing; returns (out, BassKernelResults)."""
    from concourse.bass_utils import run_bass_kernel_spmd

    nc = _get_nc()
    res = run_bass_kernel_spmd(
        nc, _in_maps(X, W1, b1, W2, b2), core_ids=list(range(B)),
        trace=True, tmpdir=tmpdir,
    )
    out = np.stack([res.results[i]["out"] for i in range(B)]).astype(np.float32)
    return out, res


# revision 36
# speedup vs baseline: 1.1396x; 1.1396x over previous
"""Trainium2 Bass kernel: causal multi-head self-attention block (B=8, T=1024, E=768, H=12).

Sharding: data-parallel over batch — one batch element per NeuronCore, 8 cores,
no collectives. Each core computes the full attention block for its batch row.

Self-contained: hardcodes all shapes; only imports concourse (installed system-wide).
"""

import numpy as np

B, T, E, H, Dh = 8, 1024, 768, 12, 64
F3 = 3 * E            # 2304
KC = E // 128         # 6 e-chunks
MT = T // 128         # 8 t-tiles
NPAIR = H // 2        # 6 head pairs
SCALE = 1.0 / float(np.sqrt(Dh))

_NC_CACHE = None


def build_nc():
    import concourse.mybir as mybir
    from concourse import bacc
    from concourse.tile import TileContext
    from concourse.masks import make_identity

    bf = mybir.dt.bfloat16
    f32 = mybir.dt.float32
    COPY = mybir.ActivationFunctionType.Copy
    EXP = mybir.ActivationFunctionType.Exp
    LN = mybir.ActivationFunctionType.Ln
    ACT_SET_LN_EXP = 6  # natural_log_exp_and_others: holds both Ln and Exp

    nc = bacc.Bacc("TRN2", target_bir_lowering=False, debug=False, num_devices=B, name="attn_dp")

    X_ext = nc.declare_dram_parameter("X", [T, E], f32, isOutput=False)
    W1_ext = nc.declare_dram_parameter("W1", [E, F3], f32, isOutput=False)
    b1_ext = nc.declare_dram_parameter("b1", [F3], f32, isOutput=False)
    W2_ext = nc.declare_dram_parameter("W2", [E, E], f32, isOutput=False)
    b2_ext = nc.declare_dram_parameter("b2", [E], f32, isOutput=False)
    out_ext = nc.declare_dram_parameter("out", [T, E], f32, isOutput=True)

    with TileContext(nc) as tc:
        with (
            tc.tile_pool(name="persist", bufs=1) as persist,
            tc.tile_pool(name="stage", bufs=2) as stage,
            tc.tile_pool(name="ptpool", bufs=10) as ptpool,
            tc.tile_pool(name="recpool", bufs=3) as recpool,
            tc.tile_pool(name="opool", bufs=2) as opool,
            tc.tile_pool(name="ps_score", bufs=1, space="PSUM") as ps_score,
            tc.tile_pool(name="ps_z", bufs=2, space="PSUM") as ps_z,
            tc.tile_pool(name="ps_acc", bufs=2, space="PSUM") as ps_acc,
        ):
            # One activation-table load for the whole kernel (covers Exp + Ln + Copy);
            # bacc's insert_act_table_loads sees coverage and adds none.
            nc.scalar.add_instruction(mybir.InstLoadActFuncSet(
                name=nc.get_next_instruction_name(), ins=[], outs=[],
                act_func_set_id=ACT_SET_LN_EXP))

            # ---- constants ----
            ident = persist.tile([128, 128], bf, tag="ident")
            make_identity(nc, ident[:])
            # multiplicative causal mask for the diagonal 128x128 block:
            # mask[k, q] = 1 where q >= k else 0
            diagmask = persist.tile([128, 128], bf, tag="diagmask")
            nc.gpsimd.memset(diagmask[:], 1.0)
            nc.gpsimd.affine_select(
                out=diagmask[:], in_=diagmask[:],
                compare_op=mybir.AluOpType.is_ge, fill=0.0, base=0,
                pattern=[[1, 128]], channel_multiplier=-1,
            )

            # per-partition bias for Q/K part of b1: b1qk[p, m] = b1[m*128 + p]
            b1qk = persist.tile([128, 12], f32, tag="b1qk")
            nc.sync.dma_start(
                out=b1qk[:], in_=b1_ext[0:1536].rearrange("(m p) -> p m", p=128)
            )
            # row biases, pre-broadcast across partitions (folded into DVE copies)
            b1v_f = stage.tile([1, E], f32, tag="rowstage")
            nc.sync.dma_start(out=b1v_f[:], in_=b1_ext[None, 1536:2304])
            b1vb = persist.tile([128, E], f32, tag="b1vb")
            nc.gpsimd.partition_broadcast(b1vb[:], b1v_f[:])
            b2_f = stage.tile([1, E], f32, tag="rowstage")
            nc.sync.dma_start(out=b2_f[:], in_=b2_ext[None, :])
            b2b = persist.tile([128, E], f32, tag="b2b")
            nc.gpsimd.partition_broadcast(b2b[:], b2_f[:])

            # ---- X: DMA f32, cast bf16 (DVE), then per-block DMA-transpose to XT[e, t] ----
            XT = persist.tile([128, KC, T], bf, tag="XT")

            def xload(mt):
                xst = stage.tile([128, E], f32, tag="xstage")
                nc.sync.dma_start(out=xst[:], in_=X_ext[mt * 128:(mt + 1) * 128, :])
                xbf = stage.tile([128, E], bf, tag="xbf")
                nc.scalar.activation(xbf[:], xst[:], COPY)
                for eg in range(KC // 2):  # pairs of e-chunks share one PSUM tile
                    pt_ps = ps_acc.tile([128, 256], bf, tag="acc")
                    nc.tensor.matmul(
                        pt_ps[:, 0:128], xbf[:, (2 * eg) * 128:(2 * eg + 1) * 128],
                        ident[:], is_transpose=True, start=True, stop=False,
                        skip_group_check=True)
                    nc.tensor.matmul(
                        pt_ps[:, 128:256], xbf[:, (2 * eg + 1) * 128:(2 * eg + 2) * 128],
                        ident[:], is_transpose=True, start=False, stop=True,
                        skip_group_check=True)
                    nc.vector.tensor_copy(
                        XT[:, 2 * eg:2 * eg + 2, mt * 128:(mt + 1) * 128],
                        pt_ps[:].rearrange("p (a n) -> p a n", a=2))

            for mt in range(MT):
                xload(mt)

            # ---- W1: DMA f32, cast to bf16 (spread across DVE/GPSIMD/ACT) ----
            W1bf = persist.tile([128, KC, F3], bf, tag="W1bf")
            for kc in range(KC):
                wst = stage.tile([128, F3], f32, tag="wstage")
                nc.sync.dma_start(out=wst[:], in_=W1_ext[kc * 128:(kc + 1) * 128, :])
                if kc % 3 == 0:
                    nc.vector.tensor_copy(W1bf[:, kc, :], wst[:])
                elif kc % 3 == 1:
                    nc.gpsimd.tensor_copy(W1bf[:, kc, :], wst[:])
                else:
                    nc.scalar.activation(W1bf[:, kc, :], wst[:], COPY)



            # QK[p, m, t]: m 0..5 = Q^T blocks (f rows m*128..), m 6..11 = K^T blocks
            QK = persist.tile([128, 12, T], bf, tag="QK")

            def qk_mtile(m):
                for nh in range(2):
                    ps = ps_acc.tile([128, 512], f32, tag="acc")
                    for kc in range(KC):
                        nc.tensor.matmul(
                            ps[:],
                            W1bf[:, kc, m * 128:(m + 1) * 128],
                            XT[:, kc, nh * 512:(nh + 1) * 512],
                            start=(kc == 0),
                            stop=(kc == KC - 1),
                        )
                    # bias (per-partition) + cast to bf16
                    nc.vector.tensor_scalar_add(
                        QK[:, m, nh * 512:(nh + 1) * 512], ps[:], b1qk[:, m:m + 1]
                    )

            qk_mtile(0)
            qk_mtile(6)

            # ---- V projection into V_aug[t-part, kt, h, 0:64] with ones col at 64 ----
            Vg = persist.tile([128, MT, H, Dh + 1], bf, tag="Vg")
            for mt in range(MT):
                nc.gpsimd.memset(Vg[:, mt, :, Dh:Dh + 1], 1.0)

            def vproj():
                for mt in range(MT):
                    for g, (n0, n) in enumerate(((0, 512), (512, 256))):
                        ps = ps_acc.tile([128, 512], f32, tag="acc")
                        for kc in range(KC):
                            nc.tensor.matmul(
                                ps[:, 0:n],
                                XT[:, kc, mt * 128:(mt + 1) * 128],
                                W1bf[:, kc, 1536 + n0:1536 + n0 + n],
                                start=(kc == 0),
                                stop=(kc == KC - 1),
                            )
                        ng = n // Dh
                        # bias add (pre-broadcast row) + cast to bf16, strided into V_aug
                        nc.vector.tensor_add(
                            Vg[:, mt, 8 * g:8 * g + ng, 0:Dh],
                            ps[:, 0:n].rearrange("p (h d) -> p h d", d=Dh),
                            b1vb[:, n0:n0 + n].rearrange("p (h d) -> p h d", d=Dh),
                        )

            # ---- attention + Z^T (divided) ----
            ZT = persist.tile([128, NPAIR, T], bf, tag="ZT")
            for hp in range(NPAIR):
                # scores + exp + mask for all kt; retain P^T tiles
                pts = []
                for kt in range(MT):
                    L = T - kt * 128
                    sc = ps_score.tile([128, 2, 1024], f32, tag="score")
                    for h01 in range(2):
                        base = h01 * 64
                        for c_off in range(0, L, 512):
                            n = min(512, L - c_off)
                            nc.tensor.matmul(
                                sc[:, h01, c_off:c_off + n],
                                QK[base:base + 64, 6 + hp, kt * 128:(kt + 1) * 128],
                                QK[base:base + 64, hp, kt * 128 + c_off:kt * 128 + c_off + n],
                                start=True,
                                stop=True,
                            )
                    ptile = ptpool.tile([128, 2, 1024], bf, tag="pt")
                    nc.scalar.activation(ptile[:, :, 0:L], sc[:, :, 0:L], EXP, scale=SCALE)
                    # causal mask on the diagonal 128x128 block (cols 0:128 of this
                    # tile): multiply by the precomputed 0/1 mask (DVE, bf16 2x)
                    for h01 in range(2):
                        nc.vector.tensor_mul(
                            ptile[:, h01, 0:128], ptile[:, h01, 0:128], diagmask[:]
                        )
                    pts.append(ptile)

                if hp == 0:
                    # V projection overlaps pair-0 exp on ACT
                    vproj()
                # emit next pair's QK projection here: overlaps with this pair's AV on PE
                if hp + 1 < NPAIR:
                    qk_mtile(hp + 1)
                    qk_mtile(6 + hp + 1)

                # attn @ V_aug, per q-chunk; denominator rides in row 64
                for c in range(2):
                    zs = []
                    for h01 in range(2):
                        z = ps_z.tile([128, 512], f32, tag="z")
                        zs.append(z)
                        h = 2 * hp + h01
                        kts = list(range(0, min(MT, 4 * (c + 1))))
                        for kt in kts:
                            zoff = max(kt * 128 - c * 512, 0)
                            n = 512 - zoff
                            poff = max(c * 512 - kt * 128, 0)
                            nc.tensor.matmul(
                                z[0:Dh + 1, zoff:zoff + n],
                                Vg[:, kt, h, 0:Dh + 1],
                                pts[kt][:, h01, poff:poff + n],
                                start=(kt == kts[0]),
                                stop=(kt == kts[-1]),
                            )
                    for h01 in range(2):
                        z = zs[h01]
                        # 1/den = exp(-ln(den)) on ScalarE; bf16 reciprocal row
                        lnden = recpool.tile([1, 512], f32, tag="lnden")
                        nc.scalar.activation(lnden[:], z[Dh:Dh + 1, :], LN)
                        rec = recpool.tile([1, 512], bf, tag="rec")
                        nc.scalar.activation(rec[:], lnden[:], EXP, scale=-1.0)
                        bc = recpool.tile([64, 512], bf, tag="bc")
                        nc.gpsimd.partition_broadcast(bc[:], rec[:])
                        nc.vector.tensor_tensor(
                            ZT[h01 * 64:(h01 + 1) * 64, hp, c * 512:(c + 1) * 512],
                            z[0:Dh, :],
                            bc[:],
                            op=mybir.AluOpType.mult,
                        )

            # ---- W2: loaded late (only needed for out-proj; overlaps attention) ----
            W2bf = persist.tile([128, KC, E], bf, tag="W2bf")
            for kc in range(KC):
                wst = stage.tile([128, F3], f32, tag="wstage")
                nc.sync.dma_start(out=wst[:, 0:E], in_=W2_ext[kc * 128:(kc + 1) * 128, :])
                if kc % 3 == 0:
                    nc.vector.tensor_copy(W2bf[:, kc, :], wst[:, 0:E])
                elif kc % 3 == 1:
                    nc.gpsimd.tensor_copy(W2bf[:, kc, :], wst[:, 0:E])
                else:
                    nc.scalar.activation(W2bf[:, kc, :], wst[:, 0:E], COPY)

            # ---- output projection: Out[t, :] = Z^T.T @ W2 + b2 ----
            for mt in range(MT):
                osb = opool.tile([128, E], f32, tag="osb")
                for g, (n0, n) in enumerate(((0, 512), (512, 256))):
                    ps = ps_acc.tile([128, 512], f32, tag="acc")
                    for pc in range(KC):
                        nc.tensor.matmul(
                            ps[:, 0:n],
                            ZT[:, pc, mt * 128:(mt + 1) * 128],
                            W2bf[:, pc, n0:n0 + n],
                            start=(pc == 0),
                            stop=(pc == KC - 1),
                        )
                    # bias add (pre-broadcast row), f32
                    nc.vector.tensor_add(osb[:, n0:n0 + n], ps[:, 0:n], b2b[:, n0:n0 + n])
                nc.sync.dma_start(out=out_ext[mt * 128:(mt + 1) * 128, :], in_=osb[:])

    nc.compile()
    return nc


def _get_nc():
    global _NC_CACHE
    if _NC_CACHE is None:
        _NC_CACHE = build_nc()
    return _NC_CACHE


def _in_maps(X, W1, b1, W2, b2):
    X = np.ascontiguousarray(np.asarray(X, dtype=np.float32))
    W1 = np.ascontiguousarray(np.asarray(W1, dtype=np.float32))
    b1 = np.ascontiguousarray(np.asarray(b1, dtype=np.float32))
    W2 = np.ascontiguousarray(np.asarray(W2, dtype=np.float32))
    b2 = np.ascontiguousarray(np.asarray(b2, dtype=np.float32))
    assert X.shape == (B, T, E)
    return [
        {"X": X[i], "W1": W1, "b1": b1, "W2": W2, "b2": b2}
        for i in range(B)
    ]


def kernel(X, W1, b1, W2, b2):
    from concourse.bass_utils import run_bass_kernel_spmd

    nc = _get_nc()
    res = run_bass_kernel_spmd(nc, _in_maps(X, W1, b1, W2, b2), core_ids=list(range(B)))
    return np.stack([res.results[i]["out"] for i in range(B)]).astype(np.float32)


def kernel_traced(X, W1, b1, W2, b2, tmpdir=None):
    """Like kernel() but with neuron-profile tracing; returns (out, BassKernelResults)."""
    from concourse.bass_utils import run_bass_kernel_spmd

    nc = _get_nc()
    res = run_bass_kernel_spmd(
        nc, _in_maps(X, W1, b1, W2, b2), core_ids=list(range(B)),
        trace=True, tmpdir=tmpdir,
    )
    out = np.stack([res.results[i]["out"] for i in range(B)]).astype(np.float32)
    return out, res
